# revision 8
# baseline (speedup 1.0000x reference)
"""Trainium2 Bass kernel for a 2-path threshold MoE router (BranchRoute).

Semantics (must match the reference):
    score = sigmoid(x @ W + b)                  # [N, 2]
    mask_p = score[:, p] >= 0.5   (== logit >= 0)
    rank_p = cumsum(mask_p) - 1                 # global pack order
    x_p[rank_p[i]] = x[i] for masked i, zero-padded to N rows
    combined[i] = (mask_0[i] + mask_1[i]) * x[i]
    returns (x0, x1, combined)

Strategy: data-parallel over tokens on 8 NeuronCores. Each core:
  - reads its 4096-token shard of x once (fp32 — the gate logits must be
    computed at full precision so the mask bits match the fp32 reference),
  - computes gate logits with fused DVE multiply+reduce ops
    (scalar_tensor_tensor) against a pre-broadcast copy of W,
  - builds per-128-token masks, in-tile ranks via a triangular matmul,
    and a running cross-tile base count via tiny PE matmuls,
  - writes `combined` and the packed dispatch buffers in bf16 (the 2e-2
    rel-err budget dwarfs bf16's ~0.2% rounding; halves write traffic),
  - compacts routed rows into its local packed buffers with indirect
    (scatter) DMAs; out-of-path rows get a huge destination index and are
    dropped by the bounds check.
Host side: per-core packed buffers are concatenated (each shard's routed
rows occupy a contiguous global range, in shard order) and upcast to fp32,
which is the unshard step for the global cumsum pack order.
"""

import numpy as np

N = 32768
D = 2048
P = 2
NCORES = 8
NLOC = N // NCORES  # 4096 tokens per core
TP = 128            # tokens per group (one SBUF partition block)
G = 2               # groups per macro tile
NMT = NLOC // (TP * G)  # macro tiles per core
BIG = 1.0e6         # offset pushed onto non-routed destinations (-> OOB drop)

_nc = None
_nc_variant = None


def _build_nc(variant="bf16", repeat=1):
    import concourse.bass as bass
    import concourse.bacc as bacc
    import concourse.tile as tile
    from concourse import mybir
    from contextlib import ExitStack

    f32 = mybir.dt.float32
    bf16 = mybir.dt.bfloat16
    i32 = mybir.dt.int32
    i16 = mybir.dt.int16

    out_dt = f32 if variant == "fp32" else bf16
    # bf16s: dispatch buffers get one extra "dump" row (NLOC) that absorbs
    # unrouted tokens; dma_scatter_add has no per-index OOB drop
    xrows = NLOC + 1 if variant == "bf16s" else NLOC
    # castdma/fakedest: no separate bf16 copy of x — the SWDGE scatter reads
    # the fp32 x tile and casts to bf16 in the DMA datapath
    inline_cast = variant in ("castdma", "fakedest")

    # Bacc (not raw Bass): its finalize() runs the lowering passes that
    # split multi-semaphore waits into standalone event-semaphore waits —
    # TRN2 instructions can carry at most one sync wait.
    nc = bacc.Bacc()

    x_h = nc.dram_tensor("x", [NLOC, D], f32, kind="ExternalInput")
    wb_h = nc.dram_tensor("wb", [TP, P, D], f32, kind="ExternalInput")
    bb_h = nc.dram_tensor("bb", [TP, P], f32, kind="ExternalInput")  # -b bcast
    tri_h = nc.dram_tensor("tribig", [TP, TP], f32, kind="ExternalInput")
    onescol_h = nc.dram_tensor("onescol", [TP, 1], f32, kind="ExternalInput")
    onesk1_h = nc.dram_tensor("onesk1", [1, TP], f32, kind="ExternalInput")
    zrow_h = nc.dram_tensor("zrow", [1, P], f32, kind="ExternalInput")
    if variant == "bf16s":
        # partition-fold selectors for the scatter_add index wrap:
        # sel16[j, q] = (j % 16 == q), ind16[j, t] = (j // 16 == t)
        sel16_h = nc.dram_tensor("sel16", [TP, 16], f32, kind="ExternalInput")
        ind16_h = nc.dram_tensor(
            "ind16", [TP, TP // 16], f32, kind="ExternalInput"
        )
    if variant == "fakedest":
        # static per-group scatter destinations (diagnostic: decouples the
        # scatter from the gate-computed ranks)
        fdest_h = nc.dram_tensor(
            "fdest", [TP, P, NMT * G], i32, kind="ExternalInput"
        )

    x0_h = nc.dram_tensor("x0", [xrows, D], out_dt, kind="ExternalOutput")
    x1_h = nc.dram_tensor("x1", [xrows, D], out_dt, kind="ExternalOutput")
    comb_h = nc.dram_tensor("comb", [NLOC, D], out_dt, kind="ExternalOutput")
    counts_h = nc.dram_tensor("counts", [1, P], i32, kind="ExternalOutput")
    xp_out = [x0_h, x1_h]

    with ExitStack() as ctx:
        tc = ctx.enter_context(tile.TileContext(nc))
        singles = ctx.enter_context(tc.tile_pool(name="singles", bufs=1))
        xp = ctx.enter_context(
            tc.tile_pool(
                name="xp",
                bufs={"bf16b": 5, "bf16s": 3, "castdma": 5, "fakedest": 5}.get(
                    variant, 4
                ),
            )
        )
        zp = ctx.enter_context(tc.tile_pool(name="zp", bufs=3))
        cp = ctx.enter_context(tc.tile_pool(name="cp", bufs=3))
        sm = ctx.enter_context(tc.tile_pool(name="sm", bufs=6))
        bp = ctx.enter_context(tc.tile_pool(name="bp", bufs=4))
        bx = ctx.enter_context(
            tc.tile_pool(name="bx", bufs=2 if variant == "bf16s" else 3)
        )
        ps = ctx.enter_context(tc.tile_pool(name="ps", bufs=2, space="PSUM"))
        pc = ctx.enter_context(
            tc.tile_pool(
                name="pc", bufs=1 if variant == "bf16s" else 2, space="PSUM"
            )
        )

        wb_sb = singles.tile([TP, P, D], f32)
        nc.sync.dma_start(out=wb_sb[:], in_=wb_h[:])
        bb_sb = singles.tile([TP, P], f32)
        nc.sync.dma_start(out=bb_sb[:], in_=bb_h[:])
        tri_sb = singles.tile([TP, TP], f32)
        nc.sync.dma_start(out=tri_sb[:], in_=tri_h[:])
        onescol_sb = singles.tile([TP, 1], f32)
        nc.sync.dma_start(out=onescol_sb[:], in_=onescol_h[:])
        onesk1_sb = singles.tile([1, TP], f32)
        nc.sync.dma_start(out=onesk1_sb[:], in_=onesk1_h[:])
        base0_sb = singles.tile([1, P], f32)
        nc.sync.dma_start(out=base0_sb[:], in_=zrow_h[:])
        if variant == "bf16s":
            sel16_sb = singles.tile([TP, 16], f32)
            nc.sync.dma_start(out=sel16_sb[:], in_=sel16_h[:])
            ind16_sb = singles.tile([TP, TP // 16], f32)
            nc.sync.dma_start(out=ind16_sb[:], in_=ind16_h[:])
        if variant == "fakedest":
            fdest_sb = singles.tile([TP, P, NMT * G], i32)
            nc.sync.dma_start(out=fdest_sb[:], in_=fdest_h[:])

        base_cur = base0_sb
        bc_reg = nc.gpsimd.to_reg(NLOC - 1)

        # Dummy matmuls so the PE consumes each constant's DMA-completion
        # semaphore once, up front. Walrus can encode only one sync-wait on
        # a Matmult (it lands on the LDWEIGHTS slot), so the real per-tile
        # matmuls must not also need to wait on these loads.
        warm_pool = pc if variant == "bf16s" else ps
        warm_ps = warm_pool.tile([TP, P], f32, tag="warm")
        nc.tensor.matmul(
            out=warm_ps[:],
            lhsT=tri_sb[:],
            rhs=tri_sb[:, 0:P],
            start=True,
            stop=True,
            skip_group_check=True,
        )
        warm_ps2 = pc.tile([1, P], f32, tag="warm2")
        nc.tensor.matmul(
            out=warm_ps2[:],
            lhsT=onescol_sb[:],
            rhs=tri_sb[:, 0:P],
            start=True,
            stop=True,
            skip_group_check=True,
        )
        warm_ps3 = warm_pool.tile([TP, P], f32, tag="warm")
        nc.tensor.matmul(
            out=warm_ps3[:],
            lhsT=onesk1_sb[:],
            rhs=onesk1_sb[:, 0:P],
            start=True,
            stop=True,
            skip_group_check=True,
        )
        if variant == "bf16s":
            warm_ps4 = warm_pool.tile([TP, P], f32, tag="warm")
            nc.tensor.matmul(
                out=warm_ps4[0:16, :],
                lhsT=sel16_sb[:],
                rhs=ind16_sb[:, 0:P],
                start=True,
                stop=True,
                skip_group_check=True,
            )

        if variant == "bf16s":
          # One dma_scatter_add per path per macro tile (256 indices — the
          # probe-validated call size; 1024-index calls drop the last row).
          # Dest indices are clamped to the dump row (min with NLOC) instead
          # of OOB-dropped; the wrapped int16 index layout is built by a PE
          # partition-fold; += lands on PJRT's pre-zeroed output buffers.
          for rep in range(repeat):
            base_cur = base0_sb
            for mt in range(NMT):
              row0 = mt * TP * G
              x_t = xp.tile([TP, G, D], f32, tag="x_t")
              nc.sync.dma_start(
                  out=x_t[:],
                  in_=x_h[row0 : row0 + TP * G, :].rearrange(
                      "(a p) d -> p a d", p=TP
                  ),
              )
              comb_t = cp.tile([TP, G, D], out_dt, tag="comb_t")
              xb_t = bx.tile([TP, G, D], bf16, tag="xb_t")
              # one [128, 16] int16 index tile per path; only partitions
              # 0-15 carry data (the wrapped layout the Q7 ucode reads)
              idxw0 = sm.tile([TP, G * (TP // 16)], i16, tag="idxw0")
              idxw1 = sm.tile([TP, G * (TP // 16)], i16, tag="idxw1")
              idxws = [idxw0, idxw1]
              for a in range(G):
                  xg = x_t[:, a, :]
                  sg = sm.tile([TP, P], f32, tag="sg")
                  for p in range(P):
                      z_t = zp.tile([TP, D], f32, tag="z_t")
                      nc.vector.scalar_tensor_tensor(
                          out=z_t[:],
                          in0=xg,
                          scalar=1.0,
                          in1=wb_sb[:, p, :],
                          op0=mybir.AluOpType.mult,
                          op1=mybir.AluOpType.mult,
                          accum_out=sg[:, p : p + 1],
                      )
                  nc.vector.tensor_copy(out=xb_t[:, a, :], in_=xg)
                  m_g = sm.tile([TP, P], f32, tag="m_g")
                  nc.vector.tensor_tensor(
                      out=m_g[:],
                      in0=sg[:],
                      in1=bb_sb[:],
                      op=mybir.AluOpType.is_ge,
                  )
                  msum = sm.tile([TP, 1], f32, tag="msum")
                  nc.vector.tensor_add(
                      out=msum[:], in0=m_g[:, 0:1], in1=m_g[:, 1:2]
                  )
                  nc.scalar.activation(
                      out=comb_t[:, a, :],
                      in_=xg,
                      func=mybir.ActivationFunctionType.Copy,
                      scale=msum[:, 0:1],
                  )
                  cnt_ps = ps.tile([1, P], f32, tag="cnt")
                  nc.tensor.matmul(
                      out=cnt_ps[:],
                      lhsT=onescol_sb[:],
                      rhs=m_g[:],
                      start=True,
                      stop=True,
                  )
                  basebig = bp.tile([1, P], f32, tag="basebig")
                  nc.vector.tensor_scalar_add(
                      out=basebig[:], in0=base_cur[:], scalar1=BIG - 1.0
                  )
                  base_next = bp.tile([1, P], f32, tag="base")
                  nc.vector.tensor_add(
                      out=base_next[:], in0=base_cur[:], in1=cnt_ps[:]
                  )
                  base_cur = base_next
                  dest_ps = ps.tile([TP, P], f32, tag="dest")
                  nc.tensor.matmul(
                      out=dest_ps[:],
                      lhsT=onesk1_sb[:],
                      rhs=basebig[:],
                      start=True,
                      stop=False,
                      skip_group_check=True,
                  )
                  nc.tensor.matmul(
                      out=dest_ps[:],
                      lhsT=tri_sb[:],
                      rhs=m_g[:],
                      start=False,
                      stop=True,
                      skip_group_check=True,
                  )
                  # partition-fold 128 dest values into the 16-partition
                  # wrapped layout: rhs_exp[j,(p,t)] = dest[j,p]*ind16[j,t],
                  # then PE-select out[q,(p,t)] = dest[16t+q, p]
                  rhs_exp = sm.tile([TP, P, TP // 16], f32, tag="rhs_exp")
                  for p in range(P):
                      nc.vector.tensor_scalar_mul(
                          out=rhs_exp[:, p, :],
                          in0=ind16_sb[:],
                          scalar1=dest_ps[:, p : p + 1],
                      )
                  wrap_ps = ps.tile([16, P, TP // 16], f32, tag="wrap")
                  nc.tensor.matmul(
                      out=wrap_ps[:],
                      lhsT=sel16_sb[:],
                      rhs=rhs_exp[:],
                      start=True,
                      stop=True,
                      skip_group_check=True,
                  )
                  # clamp unrouted (~BIG) to the dump row + cast to int16;
                  # index i of the call sits at [i%16, i//16]: column block
                  # a*8..a*8+7 holds group a
                  for p in range(P):
                      nc.vector.tensor_scalar_min(
                          out=idxws[p][
                              0:16, a * (TP // 16) : (a + 1) * (TP // 16)
                          ],
                          in0=wrap_ps[:, p, :],
                          scalar1=float(NLOC),
                      )
              nc.scalar.dma_start(
                  out=comb_h[row0 : row0 + TP * G, :].rearrange(
                      "(a p) d -> p a d", p=TP
                  ),
                  in_=comb_t[:],
              )
              for p in range(P):
                nc.gpsimd.dma_scatter_add(
                    xp_out[p][:],
                    xb_t[:],
                    idxws[p][:],
                    TP * G,
                    TP * G,
                    D,
                )

        else:
         for rep in range(repeat):
          base_cur = base0_sb
          for mt in range(NMT):
            row0 = mt * TP * G
            x_t = xp.tile([TP, G, D], f32, tag="x_t")
            nc.sync.dma_start(
                out=x_t[:],
                in_=x_h[row0 : row0 + TP * G, :].rearrange(
                    "(a p) d -> p a d", p=TP
                ),
            )
            comb_t = cp.tile([TP, G, D], out_dt, tag="comb_t")
            if variant == "bf16t":
                # dest indices live in the FREE dim of 2 partitions (one per
                # path): the Q7 SWDGE reads each group's 128 indices as one
                # contiguous 512B burst instead of 128 cross-partition reads
                dest_mt = sm.tile([P, G, TP], i32, tag="dest_mt")
            else:
                dest_mt = sm.tile([TP, P, G], i32, tag="dest_mt")
            if variant in ("bf16", "bf16t"):
                xb_t = bx.tile([TP, G, D], bf16, tag="xb_t")

            for a in range(G):
                xg = x_t[:, a, :]

                # gate logits: s[:, p] = sum_d x[tok, d] * W[d, p]
                sg = sm.tile([TP, P], f32, tag="sg")
                for p in range(P):
                    z_t = zp.tile([TP, D], f32, tag="z_t")
                    nc.vector.scalar_tensor_tensor(
                        out=z_t[:],
                        in0=xg,
                        scalar=1.0,
                        in1=wb_sb[:, p, :],
                        op0=mybir.AluOpType.mult,
                        op1=mybir.AluOpType.mult,
                        accum_out=sg[:, p : p + 1],
                    )

                # bf16 copy of x rows for the dispatch-buffer scatter
                if variant in ("bf16", "bf16t"):
                    nc.vector.tensor_copy(out=xb_t[:, a, :], in_=xg)
                    scat_src = xb_t[:, a, :]
                else:
                    # castdma/fakedest: scatter reads fp32 x directly; the
                    # SWDGE datapath casts to the bf16 output dtype
                    scat_src = xg

                # masks: m = (s + b) >= 0  <=>  s >= -b   (bb_sb holds -b)
                m_g = sm.tile([TP, P], f32, tag="m_g")
                nc.vector.tensor_tensor(
                    out=m_g[:],
                    in0=sg[:],
                    in1=bb_sb[:],
                    op=mybir.AluOpType.is_ge,
                )

                # combined = (m0 + m1) * x
                msum = sm.tile([TP, 1], f32, tag="msum")
                nc.vector.tensor_add(
                    out=msum[:], in0=m_g[:, 0:1], in1=m_g[:, 1:2]
                )
                nc.scalar.activation(
                    out=comb_t[:, a, :],
                    in_=xg,
                    func=mybir.ActivationFunctionType.Copy,
                    scale=msum[:, 0:1],
                )

                # per-group counts and running base
                cnt_ps = pc.tile([1, P], f32, tag="cnt")
                nc.tensor.matmul(
                    out=cnt_ps[:],
                    lhsT=onescol_sb[:],
                    rhs=m_g[:],
                    start=True,
                    stop=True,
                )
                basebig = bp.tile([1, P], f32, tag="basebig")
                nc.vector.tensor_scalar_add(
                    out=basebig[:], in0=base_cur[:], scalar1=BIG - 1.0
                )
                base_next = bp.tile([1, P], f32, tag="base")
                nc.vector.tensor_add(
                    out=base_next[:], in0=base_cur[:], in1=cnt_ps[:]
                )
                base_cur = base_next

                # dest = (base - 1 + BIG) + (TRI - BIG*I) @ m
                #   masked row i  -> base + (# masked j<=i) - 1   (its rank)
                #   unmasked row  -> ~BIG                          (dropped)
                if variant == "bf16t":
                    # same algebra, operands swapped: destT[p, j] lands in
                    # PSUM already transposed (indices along the free dim)
                    dest_ps = ps.tile([P, TP], f32, tag="dest")
                    nc.tensor.matmul(
                        out=dest_ps[:],
                        lhsT=basebig[:],
                        rhs=onesk1_sb[:],
                        start=True,
                        stop=False,
                        skip_group_check=True,
                    )
                    nc.tensor.matmul(
                        out=dest_ps[:],
                        lhsT=m_g[:],
                        rhs=tri_sb[:],
                        start=False,
                        stop=True,
                        skip_group_check=True,
                    )
                    nc.vector.tensor_copy(
                        out=dest_mt[:, a, :], in_=dest_ps[:]
                    )
                else:
                    dest_ps = ps.tile([TP, P], f32, tag="dest")
                    nc.tensor.matmul(
                        out=dest_ps[:],
                        lhsT=onesk1_sb[:],
                        rhs=basebig[:],
                        start=True,
                        stop=False,
                        skip_group_check=True,
                    )
                    nc.tensor.matmul(
                        out=dest_ps[:],
                        lhsT=tri_sb[:],
                        rhs=m_g[:],
                        start=False,
                        stop=True,
                        skip_group_check=True,
                    )
                    nc.vector.tensor_copy(
                        out=dest_mt[:, :, a], in_=dest_ps[:]
                    )

                if variant != "noscatter":
                    for p in range(P):
                        if variant == "fakedest":
                            off = fdest_sb[:, p, mt * G + a : mt * G + a + 1]
                        elif variant == "bf16t":
                            off = dest_mt[p : p + 1, a, :]
                        else:
                            off = dest_mt[:, p, a : a + 1]
                        nc.gpsimd.indirect_dma_start(
                            out=xp_out[p][:],
                            out_offset=bass.IndirectOffsetOnAxis(
                                ap=off, axis=0
                            ),
                            in_=scat_src,
                            in_offset=None,
                            bounds_check=bc_reg,
                            oob_is_err=False,
                        )

            # comb stores ride the ACT HWDGE ring; x loads ride SP — splits
            # load/store descriptor streams across the two HWDGE FIFOs
            nc.scalar.dma_start(
                out=comb_h[row0 : row0 + TP * G, :].rearrange(
                    "(a p) d -> p a d", p=TP
                ),
                in_=comb_t[:],
            )

        counts_i = sm.tile([1, P], i32, tag="counts_i")
        nc.vector.tensor_copy(out=counts_i[:], in_=base_cur[:])
        nc.sync.dma_start(out=counts_h[:], in_=counts_i[:])

    nc.finalize()
    return nc


def _get_nc():
    global _nc, _nc_variant
    import os

    variant = os.environ.get("BR_VARIANT", "bf16")
    repeat = int(os.environ.get("BR_REPEAT", "1"))
    key = (variant, repeat)
    if _nc is None or _nc_variant != key:
        _nc = _build_nc(variant, repeat)
        _nc_variant = key
    return _nc


def _make_const_inputs(W32, b32):
    wb = np.ascontiguousarray(
        np.broadcast_to(W32.T[None, :, :], (TP, P, D)), dtype=np.float32
    )
    bb = np.ascontiguousarray(
        np.broadcast_to(-b32[None, :], (TP, P)), dtype=np.float32
    )
    j = np.arange(TP)
    tri = (j[:, None] <= j[None, :]).astype(np.float32)
    tri[j, j] -= np.float32(BIG)
    consts = {
        "wb": wb,
        "bb": bb,
        "tribig": np.ascontiguousarray(tri),
        "onescol": np.ones((TP, 1), np.float32),
        "onesk1": np.ones((1, TP), np.float32),
        "zrow": np.zeros((1, P), np.float32),
        "sel16": np.ascontiguousarray(
            (j[:, None] % 16 == np.arange(16)[None, :]).astype(np.float32)
        ),
        "ind16": np.ascontiguousarray(
            (j[:, None] // 16 == np.arange(TP // 16)[None, :]).astype(
                np.float32
            )
        ),
    }
    # fakedest diagnostic: even tokens land at advancing dense rows, odd
    # tokens are OOB-dropped — matches the ~50% routing duty of the real gate
    t = np.arange(NMT * G)
    fd = np.where(
        (j[:, None, None] % 2) == 0,
        64 * t[None, None, :] + (j[:, None, None] // 2),
        NLOC + 7,
    ).astype(np.int32)
    consts["fdest"] = np.ascontiguousarray(
        np.broadcast_to(fd, (TP, P, NMT * G)).copy()
    )
    return consts


def run_on_cores(x, W, b, trace=False):
    """Compile (cached) + run the SPMD kernel; returns (results, bass_results)."""
    from concourse.bass_utils import run_bass_kernel_spmd

    x = np.ascontiguousarray(np.asarray(x, dtype=np.float32))
    W32 = np.ascontiguousarray(np.asarray(W, dtype=np.float32))
    b32 = np.ascontiguousarray(np.asarray(b, dtype=np.float32))
    assert x.shape == (N, D) and W32.shape == (D, P) and b32.shape == (P,)

    nc = _get_nc()
    consts = _make_const_inputs(W32, b32)
    in_maps = [
        {"x": x[c * NLOC : (c + 1) * NLOC], **consts} for c in range(NCORES)
    ]
    out = run_bass_kernel_spmd(nc, in_maps, list(range(NCORES)), trace=trace)
    return out.results, out


def kernel(x, W, b):
    results, _ = run_on_cores(x, W, b)

    x0 = np.zeros((N, D), np.float32)
    x1 = np.zeros((N, D), np.float32)
    comb = np.empty((N, D), np.float32)
    p0 = p1 = 0
    for c in range(NCORES):
        r = results[c]
        k0 = int(r["counts"][0, 0])
        k1 = int(r["counts"][0, 1])
        x0[p0 : p0 + k0] = r["x0"][:k0]
        x1[p1 : p1 + k1] = r["x1"][:k1]
        comb[c * NLOC : (c + 1) * NLOC] = r["comb"]
        p0 += k0
        p1 += k1
    return x0, x1, comb

